# revision 68
# baseline (speedup 1.0000x reference)
"""Trainium2 Bass kernel for nn_BaseSelfAttention_88433376625006.

Computes: LayerNorm -> QKV projection -> 12-head causal self-attention
(seq 4096, dim 768) -> output projection, on 8 NeuronCores.

Sharding: 4 teams x 2 cores. Team t owns heads {3t, 3t+1, 3t+2}. Within a
team, core role 0 handles query rows {0..1023, 3072..4095} and role 1 rows
{1024..3071} (equal causal work). Each core computes LN + K/V for the keys
it needs (keys are replicated inside a team), flash-style attention with the
sim matrix in [k, q] layout, and a partial output projection over its heads;
the host scatters rows and sums the 4 team partials. No collectives.

Numerics: all matmul operands are fp16 (PSUM accumulation stays f32), which
doubles PE row throughput vs f32r and halves weight-load time. LayerNorm
rstd runs as exp(-0.5*ln(var+eps)) on ACT so the only ACT table set ever
loaded is natural_log_exp (no swaps with the softmax exp). Softmax skips
max-subtraction (logits are O(1)); the denominator rides the attention
matmul as a ones-column of V and is inverted with the fast DVE reciprocal.
"""

import numpy as np

HEADS = 12
N = 4096
D = 768
DH = 64
LN_EPS = 1e-5
TEAM_HEADS = 3
HD = TEAM_HEADS * DH  # head dims per core = 192

# Role 0 owns q-tiles {0,512,3072,3584} but hands heads 1,2 of the 512-tile
# to role 1 (which already holds chunks 0-1 K/V and has spare time vs role
# 0's two extra stage-A chunks). Role 1 emits those rows as an extra
# 512-row block at the end of its output.
ROLE_SPEC = {
    0: dict(
        key_rows=4096,
        q0s=(0, 512, 3072, 3584),
        bheads={0: (0, 1, 2), 1: (0,), 2: (0, 1, 2), 3: (0, 1, 2)},
        extra=None,
        out_rows=2048,
    ),
    1: dict(
        key_rows=3072,
        q0s=(1024, 1536, 2048, 2560),
        bheads={0: (0, 1, 2), 1: (0, 1, 2), 2: (0, 1, 2), 3: (0, 1, 2), 4: (1, 2)},
        extra=dict(q0=512, qi=4, heads=(1, 2)),
        out_rows=2560,
    ),
}

_RUNNERS = None  # lazy build cache
STAGES = "ABC"  # debug: which stages to emit


# --------------------------------------------------------------------------
# neuronxcc workaround: this build rejects instructions with >1 sync wait.
# --------------------------------------------------------------------------
def _install_tile_patch():
    import concourse.tile as tile
    from concourse import mybir
    from concourse.vector_clock import ScopedClock

    if getattr(tile.TileContext, "_single_wait_patch", False):
        return

    def _patched_drain_and_barrier(self, tick_clock, wait_clock):
        nc = self.nc
        probe = nc.sync.nop(nofuse=True, hint="split_drain_waits")
        wait_clock.add_sem_waits(
            probe.ins, ScopedClock({None: tick_clock.global_clock})
        )
        si = probe.ins.sync_info
        waits = list(si.on_wait) if si and si.on_wait else []
        if len(waits) > 1:
            si.on_wait = waits[:1]
            for i in range(1, len(waits)):
                extra = nc.sync.nop(nofuse=True, hint=f"split_drain_waits_{i}")
                xsi = extra.ins.sync_info
                if xsi is None:
                    extra.ins.sync_info = mybir.SyncInfo(
                        on_wait=[waits[i]], on_update=[]
                    )
                else:
                    xsi.on_wait = [waits[i]]
        nc.sync.drain()
        nc.all_engine_barrier()
        popped = nc._tile_sem_poison_stack.pop()
        assert popped is self._sem_poison
        nc.clear_and_free_semaphores(list(self.sems.allocated().values()))
        nc.all_engine_barrier()

    tile.TileContext._drain_and_barrier = _patched_drain_and_barrier

    _orig_commit = tile.TileContext._commit_instruction

    def _patched_commit_instruction(self, inst, lazy_reg_writes=True):
        si = getattr(inst, "sync_info", None)
        if (
            si is not None
            and si.on_wait
            and len(si.on_wait) > 1
            and inst.engine != mybir.EngineType.Unassigned
        ):
            waits = list(si.on_wait)
            si.on_wait = waits[-1:]
            for w in waits[:-1]:
                nop = mybir.InstNoOp(
                    name=self.nc.get_next_instruction_name(),
                    sync_info=mybir.SyncInfo(on_wait=[w], on_update=[]),
                    bass_nofuse=True,
                    engine=inst.engine,
                )
                _orig_commit(self, nop, lazy_reg_writes=False)
        return _orig_commit(self, inst, lazy_reg_writes=lazy_reg_writes)

    tile.TileContext._commit_instruction = _patched_commit_instruction
    tile.TileContext._single_wait_patch = True


# --------------------------------------------------------------------------
# Per-device program dispatch (different programs on different cores).
# --------------------------------------------------------------------------
def _make_runner(nc):
    import jax
    from concourse import mybir
    from concourse.bass2jax import _bass_exec_p, install_neuronx_cc_hook

    install_neuronx_cc_hook()
    pid_name = nc.partition_id_tensor.name if nc.partition_id_tensor else None
    in_names, out_names, out_avals, zero_outs = [], [], [], []
    for alloc in nc.m.functions[0].allocations:
        if not isinstance(alloc, mybir.MemoryLocationSet):
            continue
        name = alloc.memorylocations[0].name
        if alloc.kind == "ExternalInput":
            if name != pid_name:
                in_names.append(name)
        elif alloc.kind == "ExternalOutput":
            shape = tuple(alloc.tensor_shape)
            dtype = mybir.dt.np(alloc.dtype)
            out_names.append(name)
            out_avals.append(jax.core.ShapedArray(shape, dtype))
            zero_outs.append(np.zeros(shape, dtype))
    n_params = len(in_names)
    all_names = in_names + out_names + ([pid_name] if pid_name else [])
    donate = tuple(range(n_params, n_params + len(out_names)))

    def _body(*args):
        return tuple(
            _bass_exec_p.bind(
                *args,
                out_avals=tuple(out_avals),
                in_names=tuple(all_names),
                out_names=tuple(out_names),
                lowering_input_output_aliases=(),
                sim_require_finite=True,
                sim_require_nnan=True,
                nc=nc,
            )
        )

    jitted = jax.jit(_body, donate_argnums=donate, keep_unused=True)
    jitted_nodonate = jax.jit(_body, keep_unused=True)

    def run(in_map, device, core_id=0):
        args = [jax.device_put(np.asarray(in_map[n]), device) for n in in_names]
        args += [jax.device_put(z.copy(), device) for z in zero_outs]
        if pid_name is not None:
            args.append(jax.device_put(np.array([[core_id]], np.uint32), device))
        outs = jitted(*args)
        return {n: outs[i] for i, n in enumerate(out_names)}

    def stage(in_map, device, core_id=0):
        args = [jax.device_put(np.asarray(in_map[n]), device) for n in in_names]
        args += [jax.device_put(z, device) for z in zero_outs]
        if pid_name is not None:
            args.append(jax.device_put(np.array([[core_id]], np.uint32), device))
        return args

    def run_staged(args):
        return jitted_nodonate(*args)

    run.stage = stage
    run.run_staged = run_staged
    run.out_names = out_names
    return run


# --------------------------------------------------------------------------
# The kernel program for one role.
# --------------------------------------------------------------------------
def _build_role_program(role, masked=False, has_bias=False):
    import concourse.bass as bass
    import concourse.tile as tile
    from concourse import mybir

    F32 = mybir.dt.float32
    F32R = mybir.dt.float32r
    F16 = mybir.dt.float16
    F8 = mybir.dt.float8e4
    DRSW = mybir.MatmulPerfMode.DoubleRowSwInterleave
    AF = mybir.ActivationFunctionType
    ALU = mybir.AluOpType

    spec = ROLE_SPEC[role]
    KR = spec["key_rows"]  # key rows this core needs
    q0s = list(spec["q0s"])  # global start row of each 512-row query tile
    bheads = spec["bheads"]  # qi -> heads this role computes attention for
    extra = spec["extra"]
    KC = KR // 512  # number of 512-row chunks
    KB = KR // 128  # number of 128-row key blocks
    q_chunks = {q0 // 512: qi for qi, q0 in enumerate(q0s)}  # chunk -> q index
    NQT = len(q0s) + (1 if extra else 0)  # tile slots incl. the extra one
    if extra:
        q0s.append(extra["q0"])
    # attention for q-tile qi can run once chunks <= q0//512 are done
    b_after = {}
    for qi in range(NQT):
        b_after.setdefault(q0s[qi] // 512, []).append(qi)

    nc = bass.Bass(enable_partition_id=False)

    x_in = nc.declare_dram_parameter("x", [KR, D], F32, isOutput=False)
    wg_in = nc.declare_dram_parameter("wg", [128, 6, 3 * HD], F16, isOutput=False)
    wv_in = nc.declare_dram_parameter("wvp", [128, 6, HD], F16, isOutput=False)
    wo_in = nc.declare_dram_parameter("wo", [128, 2304], F16, isOutput=False)
    mk_in = nc.declare_dram_parameter("maskv", [128, KB], F16, isOutput=False)
    mb_in = nc.declare_dram_parameter("mb", [128, 128], F16, isOutput=False)
    id_in = nc.declare_dram_parameter("ident", [128, 128], F16, isOutput=False)
    z8_in = nc.declare_dram_parameter("zeros8", [64, 1024], F8, isOutput=False)
    if has_bias:
        cb_in = nc.declare_dram_parameter("cb", [1, 3 * HD], F16, isOutput=False)
        cbv_in = nc.declare_dram_parameter("cbvp", [1, HD], F16, isOutput=False)
    y_out = nc.declare_dram_parameter("out", [spec["out_rows"], D], F32, isOutput=True)

    with tile.TileContext(nc) as tc:
        with (
            tc.tile_pool(name="persist", bufs=1) as pp,
            tc.tile_pool(name="work", bufs=2) as wk,
            tc.tile_pool(name="xntp", bufs=2) as xp,
            tc.tile_pool(name="xtp", bufs=5) as xtp,
            tc.tile_pool(name="small", bufs=4) as sm,
            tc.tile_pool(name="expp", bufs=4) as ep,
            tc.tile_pool(name="psga", bufs=(3 if role == 0 else 2), space="PSUM") as ps_a,
            tc.tile_pool(name="psim", bufs=2, space="PSUM") as ps_b,
            tc.tile_pool(name="pso", bufs=(1 if role == 0 else 2), space="PSUM") as ps_o,
        ):
            # ---- persistent tiles ----
            ident = pp.tile([128, 128], F16, tag="ident")
            nc.sync.dma_start(out=ident, in_=id_in[:])
            ones16 = pp.tile([1, 64], F16, tag="ones16")
            nc.vector.memset(ones16, 1.0)
            maskv = pp.tile([128, KB], F16, tag="maskv")
            nc.sync.dma_start(out=maskv, in_=mk_in[:])
            mb = pp.tile([128, 128], F16, tag="mb")
            nc.sync.dma_start(out=mb, in_=mb_in[:])
            eps_t = pp.tile([128, 1], F32, tag="eps")
            nc.vector.memset(eps_t, LN_EPS)
            # weights are fp16 in DRAM already: plain HWDGE loads, keeping
            # the casting SWDGE queue free for the chunk-0 x tiles.
            wg = pp.tile([128, 6, 3 * HD], F16, tag="wg")
            nc.sync.dma_start(out=wg, in_=wg_in[:])
            wv = pp.tile([128, 6, HD], F16, tag="wv")
            nc.sync.dma_start(out=wv, in_=wv_in[:])
            wo = pp.tile([128, 2304], F16, tag="wo")
            nc.scalar.dma_start(out=wo, in_=wo_in[:])
            if has_bias:
                cb = pp.tile([1, 3 * HD], F16, tag="cb")
                nc.sync.dma_start(out=cb, in_=cb_in[:])
                cbv = pp.tile([1, HD], F16, tag="cbv")
                nc.sync.dma_start(out=cbv, in_=cbv_in[:])
                ones16w = pp.tile([1, 512], F16, tag="ones16w")
                nc.vector.memset(ones16w, 1.0)

            # per-chunk / per-qtile persistent tiles => fine-grained deps
            # fp16 q/k feed the straddle (diagonal) sim matmuls; fp8 copies
            # feed the full key-block pairs via DoubleRowSwInterleave.
            qhh = [
                [pp.tile([64, 512], F16, name=f"qh{h}_{qi}", tag=f"qh{h}_{qi}") for qi in range(NQT)]
                for h in range(3)
            ]
            khh = [
                [pp.tile([64, 4, 128], F16, name=f"kh{h}_{c}", tag=f"kh{h}_{c}") for c in range(KC)]
                for h in range(3)
            ]
            # q8: slot0 = q, slot1 = zeros (the SwInterleave zero-slot);
            # k8: interleaved stationary, even positions 2*(127-j) hold key
            # j's kT column, odd positions (slot 1) are zeroed by a DMA fill.
            qhh8 = [
                [pp.tile([64, 1024], F8, name=f"qh8{h}_{qi}", tag=f"qh8{h}_{qi}") for qi in range(NQT)]
                for h in range(3)
            ]
            khh8 = [
                [pp.tile([64, 4, 256], F8, name=f"kh8{h}_{c}", tag=f"kh8{h}_{c}") for c in range(KC)]
                for h in range(3)
            ]
            vv = [
                pp.tile([128, 4, 3, 65], F16, name=f"vv{c}", tag=f"vv{c}")
                for c in range(KC)
            ]
            # fp8 V for pair matmuls, SwInterleave layout: dim j of slot i at
            # stored position 2*(127-j)+i, denominator (mask) at 126+i;
            # positions < 126 land on ignored psum partitions 65..127.
            vv8 = [
                pp.tile([128, 2, 3, 256], F8, name=f"vv8_{c}", tag=f"vv8_{c}")
                for c in range(KC)
            ]
            oq = [pp.tile([128, 512], F16, name=f"oq{qi}", tag=f"oq{qi}") for qi in range(NQT)]
            oq2 = [pp.tile([64, 512], F16, name=f"oq2_{qi}", tag=f"oq2_{qi}") for qi in range(NQT)]

            # psum->sbuf copies, round-robin with a per-stage ACT share.
            # (GpSimd/Pool cannot access PSUM.) ACT's Copy function lives in
            # every table set, so these never force an exp-table swap.
            # set_cp(k>0): 1/k of copies on ACT; set_cp(k<0): 1/|k| on DVE.
            _cp_state = [0, 2]

            def cp(out, in_):
                _cp_state[0] += 1
                k = _cp_state[1]
                on_act = (
                    _cp_state[0] % k == 0 if k > 0 else _cp_state[0] % (-k) != 0
                )
                if on_act:
                    nc.scalar.copy(out=out, in_=in_)
                else:
                    nc.vector.tensor_copy(out=out, in_=in_)

            def set_cp(act_every):
                _cp_state[1] = act_every

            # ---------- stage A: LN + transpose + QKV for one 512-row chunk ----
            def stage_a_chunk(c):
                set_cp(3 if role == 0 else 3)  # 1/3 of stage-A copies on ACT
                # zero-fill the fp8 k stationaries (odd = slot-1 positions
                # must be 0) and the q8 zero slots before the data writes.
                for h in range(3):
                    nc.sync.dma_start(out=khh8[h][c], in_=z8_in[:])
                qi0 = q_chunks.get(c)
                if qi0 is not None:
                    for h in bheads[qi0]:
                        nc.sync.dma_start(
                            out=qhh8[h][qi0][:, 512:1024], in_=z8_in[:, 0:512]
                        )
                if extra and c == extra["q0"] // 512:
                    for h in extra["heads"]:
                        nc.sync.dma_start(
                            out=qhh8[h][extra["qi"]][:, 512:1024], in_=z8_in[:, 0:512]
                        )
                xnT = xp.tile([128, 6, 512], F16, tag="xnT", name=f"xnT{c}")
                x_ts = []
                mvs = sm.tile([128, 4, 2], F32, tag="mvs", name=f"mvs{c}")
                for rb in range(4):
                    row0 = c * 512 + rb * 128
                    # SWDGE casting DMA: x lands in SBUF as fp16, so LN stats
                    # and the normalize run in the DVE 2x 16-bit mode.
                    x_t = xtp.tile([128, D], F16, tag="x_t", name=f"x{c}_{rb}")
                    x_ts.append(x_t)
                    nc.gpsimd.dma_start(out=x_t, in_=x_in[row0 : row0 + 128, :])
                    xr = x_t.rearrange("p (s f) -> p s f", f=256)
                    st = sm.tile([128, 3, 6], F32, tag="st", name=f"st{c}_{rb}")
                    for s in range(3):
                        nc.vector.bn_stats(out=st[:, s, :], in_=xr[:, s, :])
                    nc.vector.bn_aggr(out=mvs[:, rb, :], in_=st)
                # rstd = exp(-0.5*ln(var+eps)): stays within the ln/exp ACT
                # table set, so the softmax exp never forces a table swap.
                lnv = sm.tile([128, 4], F32, tag="lnv", name=f"lnv{c}")
                rstds = sm.tile([128, 4], F32, tag="rstds", name=f"rss{c}")
                if c == 0:  # latency-critical first chunk: per-rowblock chain
                    for rb in range(4):
                        nc.scalar.activation(
                            out=lnv[:, rb : rb + 1], in_=mvs[:, rb, 1:2],
                            func=AF.Ln, bias=eps_t, scale=1.0,
                        )
                        nc.scalar.activation(
                            out=rstds[:, rb : rb + 1], in_=lnv[:, rb : rb + 1],
                            func=AF.Exp, scale=-0.5,
                        )
                else:
                    nc.scalar.activation(
                        out=lnv, in_=mvs[:, :, 1], func=AF.Ln, bias=eps_t, scale=1.0
                    )
                    nc.scalar.activation(out=rstds, in_=lnv, func=AF.Exp, scale=-0.5)
                # xn for the latency-critical first chunk runs on ACT (idle
                # at kernel start); later chunks normalize on DVE.
                if c == 0:
                    nmrs = sm.tile([128, 4], F32, tag="nmrs", name=f"nmrs{c}")
                    nc.vector.tensor_scalar(
                        out=nmrs, in0=mvs[:, :, 0], scalar1=-1.0, scalar2=None,
                        op0=ALU.mult,
                    )
                    nc.vector.tensor_mul(out=nmrs, in0=nmrs, in1=rstds)
                for rb in range(4):
                    x_t = x_ts[rb]
                    xn = wk.tile([128, D], F16, tag="xn", name=f"xn{c}_{rb}")
                    with nc.allow_low_precision(reason="xn rounds to f16"):
                        if c == 0:
                            nc.scalar.activation(
                                out=xn, in_=x_t, func=AF.Identity,
                                bias=nmrs[:, rb : rb + 1],
                                scale=rstds[:, rb : rb + 1],
                            )
                        else:
                            nc.vector.tensor_scalar(
                                out=xn,
                                in0=x_t,
                                scalar1=mvs[:, rb, 0:1],
                                scalar2=rstds[:, rb : rb + 1],
                                op0=ALU.subtract,
                                op1=ALU.mult,
                            )
                    for half in range(2):
                        pt = ps_a.tile([128, 512], F16, tag="mma", name=f"pt{c}_{rb}_{half}")
                        for dd in range(3):
                            d = 3 * half + dd
                            nc.tensor.transpose(
                                pt[:, dd * 128 : (dd + 1) * 128],
                                xn[:, d * 128 : (d + 1) * 128],
                                ident,
                            )
                        cp(
                            xnT[:, 3 * half : 3 * half + 3, rb * 128 : (rb + 1) * 128],
                            pt[:, 0:384].rearrange("p (t f) -> p t f", f=128),
                        )

                qi = q_chunks.get(c)
                if qi is not None:
                    groups = [(0, 128, qi), (128, 256, qi), (256, 384, qi)]
                else:
                    groups = [(192, 320, None), (320, 384, None)]
                if extra and c == extra["q0"] // 512:
                    h0x = extra["heads"][0]
                    groups.append((64 * h0x, 64 * h0x + 128, extra["qi"]))
                for g0, g1, qtgt in groups:
                    gp = ps_a.tile([g1 - g0, 512], F32, tag="mma", name=f"gp{c}_{g0}")
                    for d in range(6):
                        nc.tensor.matmul(
                            gp, wg[:, d, g0:g1], xnT[:, d, :],
                            start=(d == 0), stop=(d == 5 and not has_bias),
                        )
                    if has_bias:
                        nc.tensor.matmul(gp, cb[:, g0:g1], ones16w, start=False, stop=True)
                    for s64 in range(g0, g1, 64):
                        kind, h = s64 // 192, (s64 % 192) // 64
                        sub = gp[s64 - g0 : s64 - g0 + 64, :]
                        if kind == 0:  # q: fp16 for straddles, fp8 slot0
                            cp(qhh[h][qtgt][:, :], sub)
                            cp(qhh8[h][qtgt][:, 0:512], sub)
                        elif kind == 1:  # kT: fp16 natural + fp8 interleaved
                            sub4 = sub.rearrange("p (t f) -> p t f", f=128)
                            cp(khh[h][c][:, :, :], sub4)
                            cp(khh8[h][c][:, :, 254::-2], sub4)
                        else:
                            raise AssertionError("v handled separately")
                # V in natural [key, dim] layout: xnT tiles as stationary
                for rb in range(4):
                    pvn = ps_a.tile([128, 256], F32, tag="mma", name=f"pvn{c}_{rb}")
                    for d in range(6):
                        nc.tensor.matmul(
                            pvn[:, 0:HD],
                            xnT[:, d, rb * 128 : (rb + 1) * 128],
                            wv[:, d, :],
                            start=(d == 0),
                            stop=(d == 5 and not has_bias),
                        )
                    if has_bias:
                        nc.tensor.matmul(
                            pvn[:, 0:HD], ones16w[:, 0:128], cbv, start=False, stop=True
                        )
                    v8dst = vv8[c][:, rb // 2, :, (254 + rb % 2) :: -2][:, :, 0:64]
                    if masked:
                        nc.vector.tensor_scalar_mul(
                            out=vv[c][:, rb, :, 0:64].rearrange("p h f -> p (h f)"),
                            in0=pvn[:, 0:192],
                            scalar1=maskv[:, 4 * c + rb : 4 * c + rb + 1],
                        )
                        nc.vector.tensor_copy(
                            out=v8dst, in_=vv[c][:, rb, :, 0:64]
                        )
                    else:
                        cp(vv[c][:, rb, :, 0:64], pvn[:, 0:192].rearrange("p (h f) -> p h f", f=64))
                        cp(v8dst, pvn[:, 0:192].rearrange("p (h f) -> p h f", f=64))
                for h in range(3):
                    nc.gpsimd.tensor_copy(
                        out=vv[c][:, :, h, 64], in_=maskv[:, 4 * c : 4 * c + 4]
                    )
                    nc.vector.tensor_copy(
                        out=vv8[c][:, :, h, 126:128],
                        in_=maskv[:, 4 * c : 4 * c + 4].rearrange(
                            "p (a b) -> p a b", b=2
                        ),
                    )

            # ---------- stage B: attention for one (head, q-tile) ----------
            def stage_b(h, qi):
                set_cp(-1)  # keep stage-B psum copies off ACT (exp-heavy)
                q0 = q0s[qi]
                po = ps_o.tile([128, 512], F32, tag="po", name=f"po{h}_{qi}")
                first = True
                npairs = q0 // 256
                q8mov = qhh8[h][qi][:].rearrange("p (two f) -> p two f", two=2)
                for p in range(npairs):
                    kb0 = 2 * p
                    pe_ = ps_b.tile([128, 1024], F32, tag="mmb", name=f"sp{h}_{qi}_{p}")
                    kc = kb0 // 4
                    # fp8 SwInterleave sim: slot0 = kT/q, slot1 zeroed.
                    nc.tensor.matmul(
                        pe_[:, 0:512], khh8[h][kc][:, kb0 % 4, :], q8mov,
                        start=True, stop=True, perf_mode=DRSW,
                    )
                    nc.tensor.matmul(
                        pe_[:, 512:1024], khh8[h][kc][:, kb0 % 4 + 1, :], q8mov,
                        start=True, stop=True, perf_mode=DRSW,
                    )
                    ee = ep.tile([128, 1024], F8, tag="exp", name=f"ee{h}_{qi}_{p}")
                    with nc.allow_low_precision(reason="attn weights round to f8"):
                        nc.scalar.activation(out=ee, in_=pe_, func=AF.Exp)
                    # fp8 SwInterleave attn@V: both key blocks of the pair in
                    # one matmul at half the per-row cost; V dims land on
                    # partitions 0..63, denominator on 64, junk on 65..127.
                    nc.tensor.matmul(
                        po,
                        vv8[kb0 // 4][:, (kb0 % 4) // 2, h, :],
                        ee[:].rearrange("p (two f) -> p two f", two=2),
                        start=first, stop=False, perf_mode=DRSW,
                        skip_group_check=True,
                    )
                    first = False
                # straddles: s0(512)+s1(384)+s3(128) packed in ps1; s2(256) in ps2
                kbase = q0 // 128
                ps1 = ps_b.tile([128, 1024], F32, tag="mmb", name=f"s1_{h}_{qi}")
                ps2 = ps_b.tile([128, 1024], F32, tag="mmb", name=f"s2_{h}_{qi}")
                placing = [(ps1, 0), (ps1, 512), (ps2, 0), (ps1, 896)]
                for si, (dstp, o0) in enumerate(placing):
                    r = 128 * si
                    ns = 512 - r
                    kb = kbase + si
                    qsl = qhh[h][qi][:, r:512]
                    nc.tensor.matmul(
                        dstp[:, o0 : o0 + ns],
                        khh[h][kb // 4][:, kb % 4, :],
                        qsl,
                        start=True, stop=True, skip_group_check=True,
                    )
                es1 = ep.tile([128, 1024], F16, tag="exp", name=f"e1_{h}_{qi}")
                es2 = ep.tile([128, 1024], F16, tag="exp", name=f"e2_{h}_{qi}")
                with nc.allow_low_precision(reason="attn weights round to f16"):
                    nc.scalar.activation(out=es1, in_=ps1, func=AF.Exp)
                    nc.scalar.activation(out=es2[:, 0:256], in_=ps2[:, 0:256], func=AF.Exp)
                epl = [(es1, 0), (es1, 512), (es2, 0), (es1, 896)]
                for es, o0 in epl:
                    nc.gpsimd.tensor_mul(
                        out=es[:, o0 : o0 + 128], in0=es[:, o0 : o0 + 128], in1=mb
                    )
                for si, (es, o0) in enumerate(epl):
                    r = 128 * si
                    ns = 512 - r
                    kb = kbase + si
                    nc.tensor.matmul(
                        po[0:65, r:512],
                        vv[kb // 4][:, kb % 4, h, :],
                        es[:, o0 : o0 + ns],
                        start=first, stop=(si == 3), skip_group_check=True,
                    )
                    first = False
                # normalize by denominator (row 64): fast reciprocal, then
                # broadcast down 64 partitions via a ones-column matmul.
                # 1/s = exp(-ln(s)) on ACT: ln/exp share the loaded table set.
                tln = sm.tile([1, 512], F32, tag="tln", name=f"tln{h}_{qi}")
                nc.scalar.activation(out=tln, in_=po[64:65, :], func=AF.Ln)
                rden = sm.tile([1, 512], F16, tag="rden", name=f"rd{h}_{qi}")
                with nc.allow_low_precision(reason="recip feeds PE broadcast"):
                    nc.scalar.activation(out=rden, in_=tln, func=AF.Exp, scale=-1.0)
                rdp = ps_a.tile([64, 512], F32, tag="mma", name=f"rdp{h}_{qi}")
                nc.tensor.matmul(rdp, ones16[:, 0:64], rden, start=True, stop=True)
                rdb = sm.tile([64, 512], F32, tag="rdb", name=f"rdb{h}_{qi}")
                nc.vector.tensor_copy(out=rdb, in_=rdp)
                # h1 packs down to rows 0:64 when h0 is absent (keeps the
                # later outproj operands at partition base 0).
                if h < 2:
                    row0 = 64 * h if 0 in bheads[qi] else 64 * (h - 1)
                    dst = oq[qi][row0 : row0 + 64, :]
                else:
                    dst = oq2[qi]
                with nc.allow_low_precision(reason="attn out rounds to f16"):
                    nc.vector.tensor_tensor(out=dst, in0=po[0:64, :], in1=rdb, op=ALU.mult)

            # ---------- stage C: output projection for one q-tile ----------
            def stage_c(qi):
                set_cp(2)
                heads = bheads[qi]
                for rbl in range(4):
                    rb = 4 * qi + rbl
                    sl = slice(rbl * 128, (rbl + 1) * 128)
                    # (stationary, wo row range, wo col base) per matmul;
                    # adjacent h0/h1 fuse into one 128-deep contraction.
                    parts = []
                    if 0 in heads and 1 in heads:
                        parts.append((oq[qi][:, sl], slice(0, 128), 0))
                    elif 0 in heads:
                        parts.append((oq[qi][0:64, sl], slice(0, 64), 0))
                    elif 1 in heads:
                        # h1 output lives at rows 0:64; its weights sit in
                        # the base-partition block at cols 1536:2304.
                        parts.append((oq[qi][0:64, sl], slice(0, 64), 1536))
                    if 2 in heads:
                        parts.append((oq2[qi][:, sl], slice(0, 64), 768))
                    py = ps_b.tile([128, 1024], F32, tag="mmb", name=f"py{rb}")
                    for o0 in (0, 512):
                        w = 512 if o0 == 0 else 256
                        for pi, (st, wrows, wbase) in enumerate(parts):
                            nc.tensor.matmul(
                                py[:, o0 : o0 + w],
                                st,
                                wo[wrows, wbase + o0 : wbase + o0 + w],
                                start=(pi == 0),
                                stop=(pi == len(parts) - 1),
                            )
                    y_sb = wk.tile([128, D], F32, tag="y_sb", name=f"y{rb}")
                    cp(y_sb, py[:, 0:768])
                    nc.sync.dma_start(out=y_out[rb * 128 : (rb + 1) * 128, :], in_=y_sb)

            # ---------- emission: interleave B/C into the A chunk loop ----------
            for c in range(KC):
                if "A" in STAGES:
                    stage_a_chunk(c)
                for qi in b_after.get(c, []):
                    if "B" in STAGES:
                        for h in bheads[qi]:
                            stage_b(h, qi)
                    if "C" in STAGES:
                        stage_c(qi)

    return nc


# --------------------------------------------------------------------------
# Host-side input prep
# --------------------------------------------------------------------------
def _prep_inputs(x, ln_g, ln_b, w_qkv, w_out, mask):
    x2d = np.asarray(x, np.float32).reshape(N, D)
    ln_g = np.asarray(ln_g, np.float32)
    ln_b = np.asarray(ln_b, np.float32)
    w_qkv = np.asarray(w_qkv, np.float32)
    w_out = np.asarray(w_out, np.float32)
    maskf = np.asarray(mask).reshape(N).astype(np.float32)
    scale = DH ** -0.5

    inner = HEADS * DH
    wq, wk_, wv_ = w_qkv[:, :inner], w_qkv[:, inner : 2 * inner], w_qkv[:, 2 * inner :]
    weff_q = (ln_g[:, None] * wq) * scale
    weff_k = ln_g[:, None] * wk_
    weff_v = ln_g[:, None] * wv_
    cb_q = (ln_b @ wq) * scale
    cb_k = ln_b @ wk_
    cb_v = ln_b @ wv_
    has_bias = bool(np.any(cb_q) or np.any(cb_k) or np.any(cb_v))

    mb = np.triu(np.ones((128, 128), np.float16))

    per_core = []
    for c in range(8):
        t, role = divmod(c, 2)
        spec = ROLE_SPEC[role]
        KR = spec["key_rows"]
        KB = KR // 128
        hsl = slice(3 * t * DH, (3 * t + 3) * DH)
        # [q|k|v] effective weights for this team's heads: [768, 576]
        wcat = np.concatenate(
            [weff_q[:, hsl], weff_k[:, hsl], weff_v[:, hsl]], axis=1
        )
        wg = np.ascontiguousarray(
            wcat.reshape(6, 128, 3 * HD).transpose(1, 0, 2)
        ).astype(np.float16)  # [128, 6, 576]
        wvp = (
            weff_v[:, hsl].reshape(6, 128, HD).transpose(1, 0, 2)
        ).astype(np.float16)  # [128, 6, 192]
        wo_t = w_out[hsl, :]  # [192, 768]
        wo_packed = np.zeros((128, 2304), np.float16)
        wo_packed[:, :768] = wo_t[:128]
        wo_packed[:64, 768:1536] = wo_t[128:]
        wo_packed[:64, 1536:] = wo_t[64:128]  # h1 block at base partitions
        maskv = np.ascontiguousarray(maskf[:KR].reshape(KB, 128).T).astype(
            np.float16
        )  # [128, KB]
        import ml_dtypes

        d = dict(
            x=np.ascontiguousarray(x2d[:KR]),
            wg=wg,
            wo=wo_packed,
            maskv=maskv,
            mb=mb,
            ident=np.eye(128, dtype=np.float16),
            zeros8=np.zeros((64, 1024), dtype=ml_dtypes.float8_e4m3),
            wvp=np.ascontiguousarray(wvp),
        )
        if has_bias:
            d["cb"] = np.concatenate([cb_q[hsl], cb_k[hsl], cb_v[hsl]])[None, :].astype(
                np.float16
            )
            d["cbvp"] = cb_v[hsl][None, :].astype(np.float16)
        per_core.append(d)
    return per_core, has_bias


def _get_runners(masked=False, has_bias=False):
    global _RUNNERS
    if _RUNNERS is None or _RUNNERS[2] != (masked, has_bias):
        _install_tile_patch()
        _RUNNERS = [
            _make_runner(_build_role_program(0, masked, has_bias)),
            _make_runner(_build_role_program(1, masked, has_bias)),
            (masked, has_bias),
        ]
    return _RUNNERS


def kernel(x, ln_g, ln_b, w_qkv, w_out, mask):
    import jax

    per_core, has_bias = _prep_inputs(x, ln_g, ln_b, w_qkv, w_out, mask)
    runners = _get_runners(masked=not np.asarray(mask).all(), has_bias=has_bias)
    devs = jax.devices()
    futs = [
        runners[c % 2](per_core[c], devs[c], core_id=c) for c in range(8)
    ]
    outs = [np.asarray(f["out"]) for f in futs]

    full = np.zeros((N, D), np.float32)
    for t in range(4):
        for role in (0, 1):
            o = outs[2 * t + role]
            spec = ROLE_SPEC[role]
            tiles = list(spec["q0s"])
            if spec["extra"]:
                tiles.append(spec["extra"]["q0"])
            for qi, q0 in enumerate(tiles):
                full[q0 : q0 + 512] += o[qi * 512 : (qi + 1) * 512]
    return full.reshape(np.asarray(x).shape).astype(np.float32)
